# revision 23
# baseline (speedup 1.0000x reference)
"""MoE layer (8 routed experts, top-2, shared experts) on 8 Trainium2 cores.

Strategy: expert parallelism with on-device compact dispatch, pipelined by
token halves.

Core c owns routed expert c and a 1/8 shard (MS columns) of the shared expert.
Routing is token-sharded: core r computes exact-fp32 gate logits + top-2
softmax for its 256 tokens, then an AllGather shares the [2048, 8] combine
matrix with every core. Per token-half (1024 tokens), each core compacts its
expert's routed token list with sparse_gather (capacity 384/half, tail-padded
with sentinel ids pointing past the real rows), gathers those tokens in
transposed bf16 layout from HBM with dma_gather(transpose=True), runs the
expert SwiGLU MLP on 384 tokens, scales by combine weights and
dma_scatter_adds into that half's partial buffer. The shared expert shard runs
dense (bf16, xbar DMA-transposed input) and writes the partials densely first.
Each half's partial is combined across cores by 4 chunked ReduceScatters, so
the first half's collectives overlap the second half's compute; core r ends
with tokens {ch*256 + r*32 ..+32}.

All expert/shared matmuls are bf16 (tolerance 2e-2); routing stays exact fp32.
"""

import sys

if "/opt/trn_rl_repo" not in sys.path:
    sys.path.insert(0, "/opt/trn_rl_repo")

import numpy as np

# ---- problem constants (hardcoded per contest contract) ----
B, S, H = 2, 1024, 2048
N = B * S                # 2048 tokens
NH = N // 2              # 1024 tokens per half
E = 8                    # routed experts = number of cores
M = 512                  # moe intermediate
MS_SH = 128              # shared intermediate per core (1024/8)
P = 128
KT = H // P              # 16 contraction tiles
MT = M // P              # 4 routed m-tiles
NCORES = 8
CH = 384                 # dispatch capacity per expert per half (mean 256, +9 sigma)
NPADH = NH + CH          # 1408 rows per half buffer (pad = sentinel rows)
F_WRH = NPADH // 16      # 88: per-half wrapped free size
CHS = CH // 16           # 24
CHB = CH // P            # 3 compact blocks of 128
NPAD = N + CH            # padded gather source rows (zeros at 2048+)
NSL = 256 // P           # 2 local routing slices

_CACHE = {}


def _build_program(collectives=True, loop_n=None, debug=False, sim_compat=False):
    import concourse.bass as bass
    import concourse.mybir as mybir
    import concourse.tile as tile
    from concourse import bacc
    from concourse.masks import make_identity
    from contextlib import ExitStack

    f32 = mybir.dt.float32
    bf16 = mybir.dt.bfloat16
    i16 = mybir.dt.int16
    u32 = mybir.dt.uint32
    AL = mybir.AluOpType
    ACT = mybir.ActivationFunctionType

    nc = bacc.Bacc(None)

    xhp_d = nc.declare_dram_parameter("xhp", [NPAD, H], bf16, isOutput=False)
    xs_d = nc.declare_dram_parameter("xs", [256, H], f32, isOutput=False)
    gwt_d = nc.declare_dram_parameter("gwt", [P, KT * E], f32, isOutput=False)
    wgu_d = nc.declare_dram_parameter("wgu", [MT, 2, P, KT * P], bf16, isOutput=False)
    wd_d = nc.declare_dram_parameter("wd", [P, MT, H], bf16, isOutput=False)
    swg_d = nc.declare_dram_parameter("swg", [P, KT * P], bf16, isOutput=False)
    swu_d = nc.declare_dram_parameter("swu", [P, KT * P], bf16, isOutput=False)
    swd_d = nc.declare_dram_parameter("swd", [MS_SH, H], bf16, isOutput=False)
    tokp1_d = nc.declare_dram_parameter("tokp1", [16, F_WRH], f32, isOutput=False)
    tail1_d = nc.declare_dram_parameter("tail1", [16, F_WRH], f32, isOutput=False)
    sel_d = nc.declare_dram_parameter("sel", [16, E], f32, isOutput=False)
    iota_d = nc.declare_dram_parameter("iota", [16, N // 16], i16, isOutput=False)
    if debug:
        agout_d = nc.declare_dram_parameter("agout", [N, E], f32, isOutput=False)
        agin_d = nc.declare_dram_parameter("agin", [256, E], f32, isOutput=True)
        yh0_d = nc.declare_dram_parameter("yh0", [NPADH, H], bf16, isOutput=True)
        yh1_d = nc.declare_dram_parameter("yh1", [NPADH, H], bf16, isOutput=True)
    out_d = nc.declare_dram_parameter("out", [256, H], bf16, isOutput=True)

    rg = [list(range(NCORES))]

    with tile.TileContext(nc) as tc:
        with (
            tc.tile_pool(name="sb", bufs=1) as sb,
            tc.tile_pool(name="stream", bufs=2) as st,
            tc.tile_pool(name="small", bufs=1) as sm,
            tc.tile_pool(name="ps_g", bufs=4, space="PSUM") as ps_g,
            tc.tile_pool(name="ps_y", bufs=4, space="PSUM") as ps_y,
            tc.tile_pool(name="dram", bufs=1, space="DRAM") as dram,
        ):
            if debug:
                ag_in = agin_d
                ag_out = agout_d
                y_half = [yh0_d, yh1_d]
            else:
                ag_in = dram.tile([256, E], f32, name="ag_in", tag="ag_in")
                ag_out = dram.tile([N, E], f32, name="ag_out", tag="ag_out")
                y_half = [
                    dram.tile([NPADH, H], bf16, name=f"y_h{h}", tag=f"y_h{h}")
                    for h in range(2)
                ]
            idxb = [
                dram.tile([16, CHS], i16, name=f"idxb{h}", tag=f"idxb{h}")
                for h in range(2)
            ]
            wcb = [
                dram.tile([16, CHS], f32, name=f"wcb{h}", tag=f"wcb{h}")
                for h in range(2)
            ]
            y_rs = [
                dram.tile([32, H], bf16, name=f"yrs{ch}", tag=f"yrs{ch}")
                for ch in range(8)
            ]

            ident = sb.tile([P, P], f32, name="ident")
            make_identity(nc, ident[:])
            gwt_t = sb.tile([P, KT, E], f32, name="gwt_t")
            nc.scalar.dma_start(
                gwt_t[:], gwt_d[:].rearrange("p (kt e) -> p kt e", e=E)
            )
            tokp1_t = sb.tile([16, F_WRH], f32, name="tokp1_t")
            nc.scalar.dma_start(tokp1_t[:], tokp1_d[:])
            tail1_t = sb.tile([16, F_WRH], f32, name="tail1_t")
            nc.scalar.dma_start(tail1_t[:], tail1_d[:])
            sel_t = sb.tile([16, E], f32, name="sel_t")
            nc.scalar.dma_start(sel_t[:], sel_d[:])
            swg_t = sb.tile([P, KT, P], bf16, name="swg_t")
            nc.gpsimd.dma_start(swg_t[:], swg_d[:].rearrange("p (kt m) -> p kt m", m=P))
            swu_t = sb.tile([P, KT, P], bf16, name="swu_t")
            nc.gpsimd.dma_start(swu_t[:], swu_d[:].rearrange("p (kt m) -> p kt m", m=P))
            swd_t = sb.tile([MS_SH, H], bf16, name="swd_t")
            nc.gpsimd.dma_start(swd_t[:], swd_d[:])
            wd_t = sb.tile([P, MT, H], bf16, name="wd_t")
            nc.sync.dma_start(wd_t[:], wd_d[:])
            wgu_t = sb.tile([P, MT, 2, KT, P], bf16, name="wgu_t")
            nc.sync.dma_start(
                wgu_t[:], wgu_d[:].rearrange("mt g p (kt m) -> p mt g kt m", m=P)
            )
            iota128 = sb.tile([P, N // 16], i16, name="iota128")
            nc.scalar.dma_start(
                iota128[:], iota_d[None, :, :].to_broadcast([8, 16, N // 16])
            )

            loop_ctx = ExitStack()
            if loop_n is not None:
                loop_ctx.enter_context(tc.For_i(0, loop_n, 1))

            # ================= Phase R: local routing (256 tokens, fp32) ====
            lga = sm.tile([P, NSL, E], f32, name="lga", tag="lga")
            t8 = sm.tile([P, NSL, E], f32, name="t8", tag="t8")
            for ns in range(NSL):
                x_in = st.tile([P, H], f32, name=f"xs_{ns}", tag="x_in", bufs=2)
                nc.scalar.dma_start(x_in[:], xs_d[ns * P : (ns + 1) * P, :])
                xTf32 = st.tile([P, KT, P], f32, name=f"xT32_{ns}", tag="xTf32", bufs=1)
                for g4 in range(4):
                    psA = ps_g.tile([P, 512], f32, name=f"psA_{ns}_{g4}", tag="pg")
                    for j in range(4):
                        kt = g4 * 4 + j
                        nc.tensor.transpose(
                            psA[:, j * P : (j + 1) * P],
                            x_in[:, kt * P : (kt + 1) * P],
                            ident[:],
                        )
                    nc.vector.tensor_copy(
                        xTf32[:, g4 * 4 : (g4 + 1) * 4, :],
                        psA[:, : 4 * P].rearrange("p (j c) -> p j c", j=4),
                    )
                psL = ps_y.tile([P, 512], f32, name=f"psL_{ns}", tag="py")
                for kt in range(KT):
                    nc.tensor.matmul(
                        psL[:, :E],
                        xTf32[:, kt, :],
                        gwt_t[:, kt, :],
                        start=(kt == 0),
                        stop=(kt == KT - 1),
                    )
                nc.vector.tensor_copy(lga[:, ns], psL[:, :E])
                nc.vector.max(t8[:, ns], lga[:, ns])

            # top-2 softmax combine weights for all experts, local 256 tokens
            dm = sm.tile([P, NSL], f32, name="dm", tag="rt1")
            nc.vector.tensor_tensor(dm[:], t8[:, :, 1], t8[:, :, 0], AL.subtract)
            ew = sm.tile([P, NSL], f32, name="ew", tag="rt2")
            nc.scalar.activation(ew[:], dm[:], ACT.Exp)
            z = sm.tile([P, NSL], f32, name="z", tag="rt3")
            nc.vector.tensor_scalar_add(z[:], ew[:], 1.0)
            w1 = sm.tile([P, NSL], f32, name="w1", tag="rt4")
            nc.vector.reciprocal(w1[:], z[:])
            w2 = sm.tile([P, NSL], f32, name="w2", tag="rt5")
            nc.vector.tensor_mul(w2[:], ew[:], w1[:])
            mk1 = sm.tile([P, NSL, E], f32, name="mk1", tag="rt6")
            nc.vector.tensor_tensor(
                mk1[:], lga[:], t8[:, :, 0:1].to_broadcast([P, NSL, E]), AL.is_equal
            )
            l2 = sm.tile([P, NSL, E], f32, name="l2", tag="rt7")
            nc.vector.scalar_tensor_tensor(
                l2[:], mk1[:], -1.0e30, lga[:], AL.mult, AL.add
            )
            mk2 = sm.tile([P, NSL, E], f32, name="mk2", tag="rt8")
            nc.vector.tensor_tensor(
                mk2[:], l2[:], t8[:, :, 1:2].to_broadcast([P, NSL, E]), AL.is_equal
            )
            nc.vector.tensor_tensor(
                mk1[:], mk1[:], w1[:, :, None].to_broadcast([P, NSL, E]), AL.mult
            )
            nc.vector.tensor_tensor(
                mk2[:], mk2[:], w2[:, :, None].to_broadcast([P, NSL, E]), AL.mult
            )
            comb = sm.tile([P, NSL, E], f32, name="comb", tag="rt9")
            nc.vector.tensor_add(comb[:], mk1[:], mk2[:])
            nc.sync.dma_start(
                ag_in[:].rearrange("(s p) e -> p s e", p=P), comb[:]
            )

            # ================= AllGather combine matrix =====================
            if collectives:
                nc.gpsimd.collective_compute(
                    "AllGather",
                    AL.bypass,
                    replica_groups=rg,
                    ins=[ag_in[:]],
                    outs=[ag_out[:]],
                )

            # my expert's combine column, wrapped [16, t%16 -> t//16]
            w8 = sm.tile([16, N // 16, E], f32, name="w8", tag="w8")
            nc.scalar.dma_start(w8[:], ag_out[:].rearrange("(f q) e -> q f e", q=16))
            w_wr = sm.tile([16, N // 16], f32, name="w_wr", tag="w_wr")
            nc.vector.tensor_tensor(
                w8[:], w8[:], sel_t[:, None, :].to_broadcast([16, N // 16, E]),
                AL.mult,
            )
            nc.vector.reduce_sum(w_wr[:], w8[:], axis=mybir.AxisListType.X)

            # ================= per-half: dispatch + expert + combine ========
            xTch = [None, None]
            aTch = [None, None]
            ysch = [None, None]
            idx128h = [None, None]
            wcolh = [None, None]

            def dispatch_half(h):
                w_wrh = sm.tile([16, F_WRH], f32, name=f"wwr{h}", tag=f"wwr{h}")
                nc.vector.memset(w_wrh[:], 0.0)
                nc.vector.tensor_copy(
                    w_wrh[:, : NH // 16],
                    w_wr[:, h * (NH // 16) : (h + 1) * (NH // 16)],
                )
                keep = sm.tile([16, F_WRH], f32, name=f"keep{h}", tag=f"keep{h}")
                nc.vector.tensor_scalar(keep[:], w_wrh[:], 0.0, None, AL.is_gt)
                nc.vector.tensor_tensor(keep[:], keep[:], tail1_t[:], AL.max)
                ids_m = sm.tile([16, F_WRH], f32, name=f"idsm{h}", tag=f"idsm{h}")
                nc.vector.tensor_tensor(ids_m[:], keep[:], tokp1_t[:], AL.mult)
                nc.vector.tensor_scalar_add(ids_m[:], ids_m[:], -1.0)
                w_m = sm.tile([16, F_WRH], f32, name=f"wm{h}", tag=f"wm{h}")
                nc.vector.tensor_scalar_add(w_m[:], w_wrh[:], 1.0)
                nc.vector.tensor_tensor(w_m[:], keep[:], w_m[:], AL.mult)
                nc.vector.tensor_scalar_add(w_m[:], w_m[:], -1.0)

                idc = sm.tile([16, F_WRH], f32, name=f"idc{h}", tag=f"idc{h}")
                wcc = sm.tile([16, F_WRH], f32, name=f"wcc{h}", tag=f"wcc{h}")
                nf1 = sm.tile([1, 1], u32, name=f"nf1{h}", tag=f"nf1{h}")
                nf2 = sm.tile([1, 1], u32, name=f"nf2{h}", tag=f"nf2{h}")
                nc.gpsimd.sparse_gather(idc[:], ids_m[:], num_found=nf1[:])
                nc.gpsimd.sparse_gather(wcc[:], w_m[:], num_found=nf2[:])

                # int16 local ids; bounce through DRAM to replicate to 128
                idx16 = sm.tile([16, CHS], i16, name=f"idx16{h}", tag=f"idx16{h}")
                nc.vector.tensor_copy(idx16[:], idc[:, :CHS])
                nc.scalar.dma_start(idxb[h][:], idx16[:])
                idx128h[h] = sb.tile([P, CHS], i16, name=f"idx128_{h}")
                nc.scalar.dma_start(
                    idx128h[h][:], idxb[h][None, :, :].to_broadcast([8, 16, CHS])
                )
                nc.scalar.dma_start(wcb[h][:], wcc[:, :CHS])
                wcolh[h] = sb.tile([P, CHB], f32, name=f"wcol_{h}")
                nc.scalar.dma_start(
                    wcolh[h][:], wcb[h][:].rearrange("q (c b) -> b q c", b=8)
                )

                # gather routed tokens in transposed bf16 layout
                xTch[h] = sb.tile([P, KT, CH], bf16, name=f"xTc_{h}")
                nc.gpsimd.dma_gather(
                    xTch[h][:],
                    xhp_d[h * NH : h * NH + NPADH, :],
                    idx128h[h][:],
                    CH,
                    CH,
                    H,
                    transpose=True,
                )

            def expert_half(h):
                aTch[h] = sb.tile([P, MT, CH], bf16, name=f"aTc_{h}")
                for mt in range(MT):
                    psG = ps_g.tile([P, 512], f32, name=f"psG_{h}_{mt}", tag="pg")
                    for kt in range(KT):
                        nc.tensor.matmul(
                            psG[:, :CH],
                            wgu_t[:, mt, 0, kt, :],
                            xTch[h][:, kt, :],
                            start=(kt == 0),
                            stop=(kt == KT - 1),
                        )
                    psU = ps_g.tile([P, 512], f32, name=f"psU_{h}_{mt}", tag="pg")
                    for kt in range(KT):
                        nc.tensor.matmul(
                            psU[:, :CH],
                            wgu_t[:, mt, 1, kt, :],
                            xTch[h][:, kt, :],
                            start=(kt == 0),
                            stop=(kt == KT - 1),
                        )
                    sil = sm.tile([P, CH], f32, name=f"sil_{h}_{mt}", tag="sil", bufs=2)
                    if sim_compat:
                        nc.scalar.activation(sil[:], psG[:, :CH], ACT.Sigmoid)
                        nc.vector.tensor_mul(sil[:], sil[:], psG[:, :CH])
                    else:
                        nc.scalar.activation(sil[:], psG[:, :CH], ACT.Silu)
                    nc.vector.tensor_mul(aTch[h][:, mt, :], sil[:], psU[:, :CH])

                ysch[h] = sb.tile([P, CHB, H], bf16, name=f"ysc_{h}")
                for blk in range(CHB):
                    for hq in range(4):
                        h0 = hq * 512
                        psY = ps_y.tile(
                            [P, 512], f32, name=f"psY_{h}_{blk}_{hq}", tag="py"
                        )
                        for mt in range(MT):
                            nc.tensor.matmul(
                                psY[:],
                                aTch[h][:, mt, blk * P : (blk + 1) * P],
                                wd_t[:, mt, h0 : h0 + 512],
                                start=(mt == 0),
                                stop=(mt == MT - 1),
                            )
                        nc.scalar.activation(
                            ysch[h][:, blk, h0 : h0 + 512],
                            psY[:],
                            ACT.Copy,
                            scale=wcolh[h][:, blk : blk + 1],
                        )

            def shared_gate_up():
                for ch in range(4):
                    c0 = ch * 512
                    xThi = xThi_t[ch]
                    psGs = ps_g.tile([P, 512], f32, name=f"psGs_{ch}", tag="pg")
                    for kt in range(KT):
                        nc.tensor.matmul(
                            psGs[:, :512],
                            swg_t[:, kt, :],
                            xThi[:, kt, :],
                            start=(kt == 0),
                            stop=(kt == KT - 1),
                        )
                    psUs = ps_g.tile([P, 512], f32, name=f"psUs_{ch}", tag="pg")
                    for kt in range(KT):
                        nc.tensor.matmul(
                            psUs[:, :512],
                            swu_t[:, kt, :],
                            xThi[:, kt, :],
                            start=(kt == 0),
                            stop=(kt == KT - 1),
                        )
                    sils = sm.tile([P, 512], f32, name=f"sils_{ch}", tag="sil", bufs=2)
                    if sim_compat:
                        nc.scalar.activation(sils[:], psGs[:, :512], ACT.Sigmoid)
                        nc.vector.tensor_mul(sils[:], sils[:], psGs[:, :512])
                    else:
                        nc.scalar.activation(sils[:], psGs[:, :512], ACT.Silu)
                    nc.vector.tensor_mul(
                        asT[:, c0 : c0 + 512], sils[:], psUs[:, :512]
                    )

            def shared_down_half(h):
                # shared down-proj + dense partial writes (2 slices per DMA)
                for sl2 in range(4):
                    ysh = sm.tile(
                        [P, 2, H], bf16, name=f"ysh_{h}_{sl2}", tag="ysh", bufs=2
                    )
                    for sub in range(2):
                        sl = h * 8 + sl2 * 2 + sub
                        for hq in range(4):
                            h0 = hq * 512
                            psS = ps_y.tile(
                                [P, 512], f32, name=f"psS_{sl}_{hq}", tag="py"
                            )
                            nc.tensor.matmul(
                                psS[:],
                                asT[:, sl * P : (sl + 1) * P],
                                swd_t[:, h0 : h0 + 512],
                                start=True,
                                stop=True,
                            )
                            nc.vector.tensor_copy(ysh[:, sub, h0 : h0 + 512], psS[:])
                    nc.sync.dma_start(
                        y_half[h][sl2 * 256 : (sl2 + 1) * 256, :].rearrange(
                            "(s p) e -> p s e", p=P
                        ),
                        ysh[:],
                    )

            def combine_half(h):
                for b in range(CHB):
                    nc.gpsimd.dma_scatter_add(
                        y_half[h][:],
                        ysch[h][:, b : b + 1, :],
                        idx128h[h][:, b * 8 : (b + 1) * 8],
                        P,
                        P,
                        H,
                    )
                for c4 in range(4):
                    ch = h * 4 + c4
                    if collectives:
                        nc.gpsimd.collective_compute(
                            "ReduceScatter",
                            AL.add,
                            replica_groups=rg,
                            ins=[y_half[h][c4 * 256 : (c4 + 1) * 256, :]],
                            outs=[y_rs[ch][:]],
                        )
                        eng = nc.sync if ch % 2 == 0 else nc.scalar
                        eng.dma_start(
                            out_d[ch * 32 : (ch + 1) * 32, :], y_rs[ch][:]
                        )
                    else:
                        eng = nc.sync if ch % 2 == 0 else nc.scalar
                        eng.dma_start(
                            out_d[ch * 32 : (ch + 1) * 32, :],
                            y_half[h][c4 * 256 : c4 * 256 + 32, :],
                        )

            asT = sb.tile([MS_SH, N], bf16, name="asT")

            # transposed x chunks via identity-index transpose-gathers
            xThi_t = []
            for ch in range(4):
                xt = st.tile([P, KT, 512], bf16, name=f"xThi_{ch}", tag="xThi", bufs=2)
                nc.gpsimd.dma_gather(
                    xt[:],
                    xhp_d[:],
                    iota128[:, ch * 32 : (ch + 1) * 32],
                    512,
                    512,
                    H,
                    transpose=True,
                )
                xThi_t.append(xt)

            shared_gate_up()
            dispatch_half(0)
            dispatch_half(1)
            shared_down_half(0)
            expert_half(0)
            combine_half(0)
            shared_down_half(1)
            expert_half(1)
            combine_half(1)

            loop_ctx.close()

    nc.finalize()
    return nc


def _prep_in_maps(inputs) -> list:
    import ml_dtypes

    bf = ml_dtypes.bfloat16
    x = np.asarray(inputs["hidden_states"], dtype=np.float32).reshape(N, H)
    gate_w = np.asarray(inputs["gate_w"], dtype=np.float32)
    Wg = np.asarray(inputs["Wg"], dtype=np.float32)
    Wu = np.asarray(inputs["Wu"], dtype=np.float32)
    Wd = np.asarray(inputs["Wd"], dtype=np.float32)
    sWg = np.asarray(inputs["sWg"], dtype=np.float32)
    sWu = np.asarray(inputs["sWu"], dtype=np.float32)
    sWd = np.asarray(inputs["sWd"], dtype=np.float32)

    xhp = np.zeros((NPAD, H), dtype=bf)
    xhp[:N] = x.astype(bf)
    xhp = np.ascontiguousarray(xhp)

    def tile_km(w):  # [H, Mw] -> [P, KT*Mw] with [p, kt, m] = w[kt*P+p, m]
        mw = w.shape[1]
        return np.ascontiguousarray(
            w.reshape(KT, P, mw).transpose(1, 0, 2).reshape(P, KT * mw)
        )

    def tile_km_mt(w):  # [H, M] -> [MT, P, KT*P] split by m-tile
        return np.ascontiguousarray(
            w.reshape(KT, P, MT, P).transpose(2, 1, 0, 3).reshape(MT, P, KT * P)
        )

    gwt = tile_km(np.ascontiguousarray(gate_w.T))  # [P, KT*E] fp32

    iota = np.ascontiguousarray(
        np.arange(N, dtype=np.int16).reshape(N // 16, 16).T
    )

    # per-half wrapped constants with LOCAL ids (gather src is offset per half)
    flat = np.arange(NPADH, dtype=np.float32)
    tokp1 = np.ascontiguousarray((flat + 1.0).reshape(F_WRH, 16).T)
    tail1 = np.ascontiguousarray((flat >= NH).astype(np.float32).reshape(F_WRH, 16).T)

    in_maps = []
    for c in range(NCORES):
        sel = np.zeros((16, E), dtype=np.float32)
        sel[:, c] = 1.0
        wgu = np.stack([tile_km_mt(Wg[c]), tile_km_mt(Wu[c])], axis=1)
        in_maps.append(
            {
                "xhp": xhp,
                "xs": np.ascontiguousarray(x[c * 256 : (c + 1) * 256, :]),
                "gwt": gwt,
                "wgu": np.ascontiguousarray(wgu).astype(bf),
                "wd": np.ascontiguousarray(
                    Wd[c].reshape(MT, P, H).transpose(1, 0, 2)
                ).astype(bf),
                "swg": tile_km(sWg[:, c * MS_SH : (c + 1) * MS_SH]).astype(bf),
                "swu": tile_km(sWu[:, c * MS_SH : (c + 1) * MS_SH]).astype(bf),
                "swd": np.ascontiguousarray(
                    sWd[c * MS_SH : (c + 1) * MS_SH, :]
                ).astype(bf),
                "tokp1": tokp1,
                "tail1": tail1,
                "sel": sel,
                "iota": iota,
            }
        )
    return in_maps


def _unshard(results) -> np.ndarray:
    # core r's output rows are tokens ch*256 + r*32 .. +32 for chunk ch in 0..7
    y = np.empty((N, H), dtype=np.float32)
    for r in range(NCORES):
        o = np.asarray(results[r]["out"], dtype=np.float32)  # [256, H]
        for ch in range(8):
            y[ch * 256 + r * 32 : ch * 256 + (r + 1) * 32] = o[
                ch * 32 : (ch + 1) * 32
            ]
    return y.reshape(B, S, H)


def kernel(**inputs) -> np.ndarray:
    from concourse.bass_utils import run_bass_kernel_spmd

    in_maps = _prep_in_maps(inputs)

    if "nc" not in _CACHE:
        _CACHE["nc"] = _build_program()
    nc = _CACHE["nc"]

    res = run_bass_kernel_spmd(nc, in_maps, list(range(NCORES))).results
    return _unshard(res)


if __name__ == "__main__":
    # smoke test against the local reference
    sys.path.insert(0, "/root/problem")
    import reference

    inp = reference.setup_inputs()
    expected = np.asarray(reference.reference(**inp))
    actual = kernel(**{k: np.asarray(v) for k, v in inp.items()})
    err = np.linalg.norm(actual - expected) / np.linalg.norm(expected)
    print("Relative error:", err)


# revision 24
# speedup vs baseline: 1.1264x; 1.1264x over previous
"""MoE layer (8 routed experts, top-2, shared experts) on 8 Trainium2 cores.

Strategy: expert parallelism with on-device compact dispatch, pipelined by
token halves.

Core c owns routed expert c and a 1/8 shard (MS columns) of the shared expert.
Routing is token-sharded: core r computes exact-fp32 gate logits + top-2
softmax for its 256 tokens, then an AllGather shares the [2048, 8] combine
matrix with every core. Per token-half (1024 tokens), each core compacts its
expert's routed token list with sparse_gather (capacity 384/half, tail-padded
with sentinel ids pointing past the real rows), gathers those tokens in
transposed bf16 layout from HBM with dma_gather(transpose=True), runs the
expert SwiGLU MLP on 384 tokens, scales by combine weights and
dma_scatter_adds into that half's partial buffer. The shared expert shard runs
dense (bf16, xbar DMA-transposed input) and writes the partials densely first.
Each half's partial is combined across cores by 4 chunked ReduceScatters, so
the first half's collectives overlap the second half's compute; core r ends
with tokens {ch*256 + r*32 ..+32}.

All expert/shared matmuls are bf16 (tolerance 2e-2); routing stays exact fp32.
"""

import sys

if "/opt/trn_rl_repo" not in sys.path:
    sys.path.insert(0, "/opt/trn_rl_repo")

import numpy as np

# ---- problem constants (hardcoded per contest contract) ----
B, S, H = 2, 1024, 2048
N = B * S                # 2048 tokens
NH = N // 2              # 1024 tokens per half
E = 8                    # routed experts = number of cores
M = 512                  # moe intermediate
MS_SH = 128              # shared intermediate per core (1024/8)
P = 128
KT = H // P              # 16 contraction tiles
MT = M // P              # 4 routed m-tiles
NCORES = 8
CH = 384                 # dispatch capacity per expert per half (mean 256, +9 sigma)
NPADH = NH + CH          # 1408 rows per half buffer (pad = sentinel rows)
F_WRH = NPADH // 16      # 88: per-half wrapped free size
CHS = CH // 16           # 24
CHB = CH // P            # 3 compact blocks of 128
NPAD = N + CH            # padded gather source rows (zeros at 2048+)
NSL = 256 // P           # 2 local routing slices

_CACHE = {}


def _build_program(collectives=True, loop_n=None, debug=False, sim_compat=False):
    import concourse.bass as bass
    import concourse.mybir as mybir
    import concourse.tile as tile
    from concourse import bacc
    from concourse.masks import make_identity
    from contextlib import ExitStack

    f32 = mybir.dt.float32
    bf16 = mybir.dt.bfloat16
    i16 = mybir.dt.int16
    u32 = mybir.dt.uint32
    AL = mybir.AluOpType
    ACT = mybir.ActivationFunctionType

    nc = bacc.Bacc(None)

    xhp_d = nc.declare_dram_parameter("xhp", [NPAD, H], bf16, isOutput=False)
    xs_d = nc.declare_dram_parameter("xs", [256, H], f32, isOutput=False)
    gwt_d = nc.declare_dram_parameter("gwt", [P, KT * E], f32, isOutput=False)
    wgu_d = nc.declare_dram_parameter("wgu", [MT, 2, P, KT * P], bf16, isOutput=False)
    wd_d = nc.declare_dram_parameter("wd", [P, MT, H], bf16, isOutput=False)
    swg_d = nc.declare_dram_parameter("swg", [P, KT * P], bf16, isOutput=False)
    swu_d = nc.declare_dram_parameter("swu", [P, KT * P], bf16, isOutput=False)
    swd_d = nc.declare_dram_parameter("swd", [MS_SH, H], bf16, isOutput=False)
    tokp1_d = nc.declare_dram_parameter("tokp1", [16, F_WRH], f32, isOutput=False)
    tail1_d = nc.declare_dram_parameter("tail1", [16, F_WRH], f32, isOutput=False)
    sel_d = nc.declare_dram_parameter("sel", [16, E], f32, isOutput=False)
    iota_d = nc.declare_dram_parameter("iota", [16, N // 16], i16, isOutput=False)
    if debug:
        agout_d = nc.declare_dram_parameter("agout", [N, E], f32, isOutput=False)
        agin_d = nc.declare_dram_parameter("agin", [256, E], f32, isOutput=True)
        yh0_d = nc.declare_dram_parameter("yh0", [NPADH, H], bf16, isOutput=True)
        yh1_d = nc.declare_dram_parameter("yh1", [NPADH, H], bf16, isOutput=True)
    out_d = nc.declare_dram_parameter("out", [256, H], bf16, isOutput=True)

    rg = [list(range(NCORES))]

    with tile.TileContext(nc) as tc:
        with (
            tc.tile_pool(name="sb", bufs=1) as sb,
            tc.tile_pool(name="stream", bufs=2) as st,
            tc.tile_pool(name="small", bufs=1) as sm,
            tc.tile_pool(name="ps_g", bufs=4, space="PSUM") as ps_g,
            tc.tile_pool(name="ps_y", bufs=4, space="PSUM") as ps_y,
            tc.tile_pool(name="dram", bufs=1, space="DRAM") as dram,
        ):
            if debug:
                ag_in = agin_d
                ag_out = agout_d
                y_half = [yh0_d, yh1_d]
            else:
                ag_in = dram.tile([256, E], f32, name="ag_in", tag="ag_in")
                ag_out = dram.tile([N, E], f32, name="ag_out", tag="ag_out")
                y_half = [
                    dram.tile([NPADH, H], bf16, name=f"y_h{h}", tag=f"y_h{h}")
                    for h in range(2)
                ]
            idxb = [
                dram.tile([16, CHS], i16, name=f"idxb{h}", tag=f"idxb{h}")
                for h in range(2)
            ]
            wcb = [
                dram.tile([16, CHS], f32, name=f"wcb{h}", tag=f"wcb{h}")
                for h in range(2)
            ]
            y_rs = [
                dram.tile([32, H], bf16, name=f"yrs{ch}", tag=f"yrs{ch}")
                for ch in range(8)
            ]

            ident = sb.tile([P, P], f32, name="ident")
            make_identity(nc, ident[:])
            gwt_t = sb.tile([P, KT, E], f32, name="gwt_t")
            nc.scalar.dma_start(
                gwt_t[:], gwt_d[:].rearrange("p (kt e) -> p kt e", e=E)
            )
            tokp1_t = sb.tile([16, F_WRH], f32, name="tokp1_t")
            nc.scalar.dma_start(tokp1_t[:], tokp1_d[:])
            tail1_t = sb.tile([16, F_WRH], f32, name="tail1_t")
            nc.scalar.dma_start(tail1_t[:], tail1_d[:])
            sel_t = sb.tile([16, E], f32, name="sel_t")
            nc.scalar.dma_start(sel_t[:], sel_d[:])
            swg_t = sb.tile([P, KT, P], bf16, name="swg_t")
            nc.gpsimd.dma_start(swg_t[:], swg_d[:].rearrange("p (kt m) -> p kt m", m=P))
            swu_t = sb.tile([P, KT, P], bf16, name="swu_t")
            nc.gpsimd.dma_start(swu_t[:], swu_d[:].rearrange("p (kt m) -> p kt m", m=P))
            swd_t = sb.tile([MS_SH, H], bf16, name="swd_t")
            nc.gpsimd.dma_start(swd_t[:], swd_d[:])
            wd_t = sb.tile([P, MT, H], bf16, name="wd_t")
            nc.sync.dma_start(wd_t[:], wd_d[:])
            wgu_t = sb.tile([P, MT, 2, KT, P], bf16, name="wgu_t")
            nc.sync.dma_start(
                wgu_t[:], wgu_d[:].rearrange("mt g p (kt m) -> p mt g kt m", m=P)
            )
            iota128 = sb.tile([P, N // 16], i16, name="iota128")
            nc.scalar.dma_start(
                iota128[:], iota_d[None, :, :].to_broadcast([8, 16, N // 16])
            )

            loop_ctx = ExitStack()
            if loop_n is not None:
                loop_ctx.enter_context(tc.For_i(0, loop_n, 1))

            # ================= Phase R: local routing (256 tokens, fp32) ====
            lga = sm.tile([P, NSL, E], f32, name="lga", tag="lga")
            t8 = sm.tile([P, NSL, E], f32, name="t8", tag="t8")
            for ns in range(NSL):
                x_in = st.tile([P, H], f32, name=f"xs_{ns}", tag="x_in", bufs=2)
                nc.scalar.dma_start(x_in[:], xs_d[ns * P : (ns + 1) * P, :])
                xTf32 = st.tile([P, KT, P], f32, name=f"xT32_{ns}", tag="xTf32", bufs=1)
                for g4 in range(4):
                    psA = ps_g.tile([P, 512], f32, name=f"psA_{ns}_{g4}", tag="pg")
                    for j in range(4):
                        kt = g4 * 4 + j
                        nc.tensor.transpose(
                            psA[:, j * P : (j + 1) * P],
                            x_in[:, kt * P : (kt + 1) * P],
                            ident[:],
                        )
                    nc.vector.tensor_copy(
                        xTf32[:, g4 * 4 : (g4 + 1) * 4, :],
                        psA[:, : 4 * P].rearrange("p (j c) -> p j c", j=4),
                    )
                psL = ps_y.tile([P, 512], f32, name=f"psL_{ns}", tag="py")
                for kt in range(KT):
                    nc.tensor.matmul(
                        psL[:, :E],
                        xTf32[:, kt, :],
                        gwt_t[:, kt, :],
                        start=(kt == 0),
                        stop=(kt == KT - 1),
                    )
                nc.vector.tensor_copy(lga[:, ns], psL[:, :E])
                nc.vector.max(t8[:, ns], lga[:, ns])

            # top-2 softmax combine weights for all experts, local 256 tokens
            dm = sm.tile([P, NSL], f32, name="dm", tag="rt1")
            nc.vector.tensor_tensor(dm[:], t8[:, :, 1], t8[:, :, 0], AL.subtract)
            ew = sm.tile([P, NSL], f32, name="ew", tag="rt2")
            nc.scalar.activation(ew[:], dm[:], ACT.Exp)
            z = sm.tile([P, NSL], f32, name="z", tag="rt3")
            nc.vector.tensor_scalar_add(z[:], ew[:], 1.0)
            w1 = sm.tile([P, NSL], f32, name="w1", tag="rt4")
            nc.vector.reciprocal(w1[:], z[:])
            w2 = sm.tile([P, NSL], f32, name="w2", tag="rt5")
            nc.vector.tensor_mul(w2[:], ew[:], w1[:])
            mk1 = sm.tile([P, NSL, E], f32, name="mk1", tag="rt6")
            nc.vector.tensor_tensor(
                mk1[:], lga[:], t8[:, :, 0:1].to_broadcast([P, NSL, E]), AL.is_equal
            )
            l2 = sm.tile([P, NSL, E], f32, name="l2", tag="rt7")
            nc.vector.scalar_tensor_tensor(
                l2[:], mk1[:], -1.0e30, lga[:], AL.mult, AL.add
            )
            mk2 = sm.tile([P, NSL, E], f32, name="mk2", tag="rt8")
            nc.vector.tensor_tensor(
                mk2[:], l2[:], t8[:, :, 1:2].to_broadcast([P, NSL, E]), AL.is_equal
            )
            nc.vector.tensor_tensor(
                mk1[:], mk1[:], w1[:, :, None].to_broadcast([P, NSL, E]), AL.mult
            )
            nc.vector.tensor_tensor(
                mk2[:], mk2[:], w2[:, :, None].to_broadcast([P, NSL, E]), AL.mult
            )
            comb = sm.tile([P, NSL, E], f32, name="comb", tag="rt9")
            nc.vector.tensor_add(comb[:], mk1[:], mk2[:])
            nc.sync.dma_start(
                ag_in[:].rearrange("(s p) e -> p s e", p=P), comb[:]
            )

            # ================= AllGather combine matrix =====================
            if collectives:
                nc.gpsimd.collective_compute(
                    "AllGather",
                    AL.bypass,
                    replica_groups=rg,
                    ins=[ag_in[:]],
                    outs=[ag_out[:]],
                )

            # my expert's combine column, wrapped [16, t%16 -> t//16]
            w8 = sm.tile([16, N // 16, E], f32, name="w8", tag="w8")
            nc.scalar.dma_start(w8[:], ag_out[:].rearrange("(f q) e -> q f e", q=16))
            w_wr = sm.tile([16, N // 16], f32, name="w_wr", tag="w_wr")
            nc.vector.tensor_tensor(
                w8[:], w8[:], sel_t[:, None, :].to_broadcast([16, N // 16, E]),
                AL.mult,
            )
            nc.vector.reduce_sum(w_wr[:], w8[:], axis=mybir.AxisListType.X)

            # ================= per-half: dispatch + expert + combine ========
            xTch = [None, None]
            aTch = [None, None]
            ysch = [None, None]
            idx128h = [None, None]
            wcolh = [None, None]

            def dispatch_half(h):
                w_wrh = sm.tile([16, F_WRH], f32, name=f"wwr{h}", tag=f"wwr{h}")
                nc.vector.memset(w_wrh[:], 0.0)
                nc.vector.tensor_copy(
                    w_wrh[:, : NH // 16],
                    w_wr[:, h * (NH // 16) : (h + 1) * (NH // 16)],
                )
                keep = sm.tile([16, F_WRH], f32, name=f"keep{h}", tag=f"keep{h}")
                nc.vector.tensor_scalar(keep[:], w_wrh[:], 0.0, None, AL.is_gt)
                nc.vector.tensor_tensor(keep[:], keep[:], tail1_t[:], AL.max)
                ids_m = sm.tile([16, F_WRH], f32, name=f"idsm{h}", tag=f"idsm{h}")
                nc.vector.tensor_tensor(ids_m[:], keep[:], tokp1_t[:], AL.mult)
                nc.vector.tensor_scalar_add(ids_m[:], ids_m[:], -1.0)
                w_m = sm.tile([16, F_WRH], f32, name=f"wm{h}", tag=f"wm{h}")
                nc.vector.tensor_scalar_add(w_m[:], w_wrh[:], 1.0)
                nc.vector.tensor_tensor(w_m[:], keep[:], w_m[:], AL.mult)
                nc.vector.tensor_scalar_add(w_m[:], w_m[:], -1.0)

                idc = sm.tile([16, F_WRH], f32, name=f"idc{h}", tag=f"idc{h}")
                wcc = sm.tile([16, F_WRH], f32, name=f"wcc{h}", tag=f"wcc{h}")
                nf1 = sm.tile([1, 1], u32, name=f"nf1{h}", tag=f"nf1{h}")
                nf2 = sm.tile([1, 1], u32, name=f"nf2{h}", tag=f"nf2{h}")
                nc.gpsimd.sparse_gather(idc[:], ids_m[:], num_found=nf1[:])
                nc.gpsimd.sparse_gather(wcc[:], w_m[:], num_found=nf2[:])

                # int16 local ids; bounce through DRAM to replicate to 128
                idx16 = sm.tile([16, CHS], i16, name=f"idx16{h}", tag=f"idx16{h}")
                nc.vector.tensor_copy(idx16[:], idc[:, :CHS])
                nc.scalar.dma_start(idxb[h][:], idx16[:])
                idx128h[h] = sb.tile([P, CHS], i16, name=f"idx128_{h}")
                nc.scalar.dma_start(
                    idx128h[h][:], idxb[h][None, :, :].to_broadcast([8, 16, CHS])
                )
                nc.scalar.dma_start(wcb[h][:], wcc[:, :CHS])
                wcolh[h] = sb.tile([P, CHB], f32, name=f"wcol_{h}")
                nc.scalar.dma_start(
                    wcolh[h][:], wcb[h][:].rearrange("q (c b) -> b q c", b=8)
                )

                # gather routed tokens in transposed bf16 layout
                xTch[h] = sb.tile([P, KT, CH], bf16, name=f"xTc_{h}")
                nc.gpsimd.dma_gather(
                    xTch[h][:],
                    xhp_d[h * NH : h * NH + NPADH, :],
                    idx128h[h][:],
                    CH,
                    CH,
                    H,
                    transpose=True,
                )

            def expert_half(h):
                aTch[h] = sb.tile([P, MT, CH], bf16, name=f"aTc_{h}")
                for mt in range(MT):
                    psG = ps_g.tile([P, 512], f32, name=f"psG_{h}_{mt}", tag="pg")
                    for kt in range(KT):
                        nc.tensor.matmul(
                            psG[:, :CH],
                            wgu_t[:, mt, 0, kt, :],
                            xTch[h][:, kt, :],
                            start=(kt == 0),
                            stop=(kt == KT - 1),
                        )
                    psU = ps_g.tile([P, 512], f32, name=f"psU_{h}_{mt}", tag="pg")
                    for kt in range(KT):
                        nc.tensor.matmul(
                            psU[:, :CH],
                            wgu_t[:, mt, 1, kt, :],
                            xTch[h][:, kt, :],
                            start=(kt == 0),
                            stop=(kt == KT - 1),
                        )
                    sil = sm.tile([P, CH], f32, name=f"sil_{h}_{mt}", tag="sil", bufs=2)
                    if sim_compat:
                        nc.scalar.activation(sil[:], psG[:, :CH], ACT.Sigmoid)
                        nc.vector.tensor_mul(sil[:], sil[:], psG[:, :CH])
                    else:
                        nc.scalar.activation(sil[:], psG[:, :CH], ACT.Silu)
                    nc.vector.tensor_mul(aTch[h][:, mt, :], sil[:], psU[:, :CH])

                ysch[h] = sb.tile([P, CHB, H], bf16, name=f"ysc_{h}")
                for blk in range(CHB):
                    for hq in range(4):
                        h0 = hq * 512
                        psY = ps_y.tile(
                            [P, 512], f32, name=f"psY_{h}_{blk}_{hq}", tag="py"
                        )
                        for mt in range(MT):
                            nc.tensor.matmul(
                                psY[:],
                                aTch[h][:, mt, blk * P : (blk + 1) * P],
                                wd_t[:, mt, h0 : h0 + 512],
                                start=(mt == 0),
                                stop=(mt == MT - 1),
                            )
                        nc.scalar.activation(
                            ysch[h][:, blk, h0 : h0 + 512],
                            psY[:],
                            ACT.Copy,
                            scale=wcolh[h][:, blk : blk + 1],
                        )

            def shared_gate_up():
                for ch in range(4):
                    c0 = ch * 512
                    xThi = xThi_t[ch]
                    psGs = ps_g.tile([P, 512], f32, name=f"psGs_{ch}", tag="pg")
                    for kt in range(KT):
                        nc.tensor.matmul(
                            psGs[:, :512],
                            swg_t[:, kt, :],
                            xThi[:, kt, :],
                            start=(kt == 0),
                            stop=(kt == KT - 1),
                        )
                    psUs = ps_g.tile([P, 512], f32, name=f"psUs_{ch}", tag="pg")
                    for kt in range(KT):
                        nc.tensor.matmul(
                            psUs[:, :512],
                            swu_t[:, kt, :],
                            xThi[:, kt, :],
                            start=(kt == 0),
                            stop=(kt == KT - 1),
                        )
                    sils = sm.tile([P, 512], f32, name=f"sils_{ch}", tag="sil", bufs=2)
                    if sim_compat:
                        nc.scalar.activation(sils[:], psGs[:, :512], ACT.Sigmoid)
                        nc.vector.tensor_mul(sils[:], sils[:], psGs[:, :512])
                    else:
                        nc.scalar.activation(sils[:], psGs[:, :512], ACT.Silu)
                    nc.vector.tensor_mul(
                        asT[:, c0 : c0 + 512], sils[:], psUs[:, :512]
                    )

            def shared_down_half(h):
                # shared down-proj + dense partial writes (2 slices per DMA)
                for sl2 in range(4):
                    ysh = sm.tile(
                        [P, 2, H], bf16, name=f"ysh_{h}_{sl2}", tag="ysh", bufs=2
                    )
                    for sub in range(2):
                        sl = h * 8 + sl2 * 2 + sub
                        for hq in range(4):
                            h0 = hq * 512
                            psS = ps_y.tile(
                                [P, 512], f32, name=f"psS_{sl}_{hq}", tag="py"
                            )
                            nc.tensor.matmul(
                                psS[:],
                                asT[:, sl * P : (sl + 1) * P],
                                swd_t[:, h0 : h0 + 512],
                                start=True,
                                stop=True,
                            )
                            nc.vector.tensor_copy(ysh[:, sub, h0 : h0 + 512], psS[:])
                    nc.sync.dma_start(
                        y_half[h][sl2 * 256 : (sl2 + 1) * 256, :].rearrange(
                            "(s p) e -> p s e", p=P
                        ),
                        ysh[:],
                    )

            def combine_half(h):
                for b in range(CHB):
                    nc.gpsimd.dma_scatter_add(
                        y_half[h][:],
                        ysch[h][:, b : b + 1, :],
                        idx128h[h][:, b * 8 : (b + 1) * 8],
                        P,
                        P,
                        H,
                    )
                for c4 in range(4):
                    ch = h * 4 + c4
                    if collectives:
                        nc.gpsimd.collective_compute(
                            "ReduceScatter",
                            AL.add,
                            replica_groups=rg,
                            ins=[y_half[h][c4 * 256 : (c4 + 1) * 256, :]],
                            outs=[y_rs[ch][:]],
                        )
                        eng = nc.sync if ch % 2 == 0 else nc.scalar
                        eng.dma_start(
                            out_d[ch * 32 : (ch + 1) * 32, :], y_rs[ch][:]
                        )
                    else:
                        eng = nc.sync if ch % 2 == 0 else nc.scalar
                        eng.dma_start(
                            out_d[ch * 32 : (ch + 1) * 32, :],
                            y_half[h][c4 * 256 : c4 * 256 + 32, :],
                        )

            asT = sb.tile([MS_SH, N], bf16, name="asT")

            # transposed x chunks via identity-index transpose-gathers
            xThi_t = []
            for ch in range(4):
                xt = st.tile([P, KT, 512], bf16, name=f"xThi_{ch}", tag="xThi", bufs=2)
                nc.gpsimd.dma_gather(
                    xt[:],
                    xhp_d[:],
                    iota128[:, ch * 32 : (ch + 1) * 32],
                    512,
                    512,
                    H,
                    transpose=True,
                )
                xThi_t.append(xt)

            dispatch_half(0)
            dispatch_half(1)
            shared_gate_up()
            shared_down_half(0)
            expert_half(0)
            combine_half(0)
            shared_down_half(1)
            expert_half(1)
            combine_half(1)

            loop_ctx.close()

    nc.finalize()
    return nc


def _prep_in_maps(inputs) -> list:
    import ml_dtypes

    bf = ml_dtypes.bfloat16
    x = np.asarray(inputs["hidden_states"], dtype=np.float32).reshape(N, H)
    gate_w = np.asarray(inputs["gate_w"], dtype=np.float32)
    Wg = np.asarray(inputs["Wg"], dtype=np.float32)
    Wu = np.asarray(inputs["Wu"], dtype=np.float32)
    Wd = np.asarray(inputs["Wd"], dtype=np.float32)
    sWg = np.asarray(inputs["sWg"], dtype=np.float32)
    sWu = np.asarray(inputs["sWu"], dtype=np.float32)
    sWd = np.asarray(inputs["sWd"], dtype=np.float32)

    xhp = np.zeros((NPAD, H), dtype=bf)
    xhp[:N] = x.astype(bf)
    xhp = np.ascontiguousarray(xhp)

    def tile_km(w):  # [H, Mw] -> [P, KT*Mw] with [p, kt, m] = w[kt*P+p, m]
        mw = w.shape[1]
        return np.ascontiguousarray(
            w.reshape(KT, P, mw).transpose(1, 0, 2).reshape(P, KT * mw)
        )

    def tile_km_mt(w):  # [H, M] -> [MT, P, KT*P] split by m-tile
        return np.ascontiguousarray(
            w.reshape(KT, P, MT, P).transpose(2, 1, 0, 3).reshape(MT, P, KT * P)
        )

    gwt = tile_km(np.ascontiguousarray(gate_w.T))  # [P, KT*E] fp32

    iota = np.ascontiguousarray(
        np.arange(N, dtype=np.int16).reshape(N // 16, 16).T
    )

    # per-half wrapped constants with LOCAL ids (gather src is offset per half)
    flat = np.arange(NPADH, dtype=np.float32)
    tokp1 = np.ascontiguousarray((flat + 1.0).reshape(F_WRH, 16).T)
    tail1 = np.ascontiguousarray((flat >= NH).astype(np.float32).reshape(F_WRH, 16).T)

    in_maps = []
    for c in range(NCORES):
        sel = np.zeros((16, E), dtype=np.float32)
        sel[:, c] = 1.0
        wgu = np.stack([tile_km_mt(Wg[c]), tile_km_mt(Wu[c])], axis=1)
        in_maps.append(
            {
                "xhp": xhp,
                "xs": np.ascontiguousarray(x[c * 256 : (c + 1) * 256, :]),
                "gwt": gwt,
                "wgu": np.ascontiguousarray(wgu).astype(bf),
                "wd": np.ascontiguousarray(
                    Wd[c].reshape(MT, P, H).transpose(1, 0, 2)
                ).astype(bf),
                "swg": tile_km(sWg[:, c * MS_SH : (c + 1) * MS_SH]).astype(bf),
                "swu": tile_km(sWu[:, c * MS_SH : (c + 1) * MS_SH]).astype(bf),
                "swd": np.ascontiguousarray(
                    sWd[c * MS_SH : (c + 1) * MS_SH, :]
                ).astype(bf),
                "tokp1": tokp1,
                "tail1": tail1,
                "sel": sel,
                "iota": iota,
            }
        )
    return in_maps


def _unshard(results) -> np.ndarray:
    # core r's output rows are tokens ch*256 + r*32 .. +32 for chunk ch in 0..7
    y = np.empty((N, H), dtype=np.float32)
    for r in range(NCORES):
        o = np.asarray(results[r]["out"], dtype=np.float32)  # [256, H]
        for ch in range(8):
            y[ch * 256 + r * 32 : ch * 256 + (r + 1) * 32] = o[
                ch * 32 : (ch + 1) * 32
            ]
    return y.reshape(B, S, H)


def kernel(**inputs) -> np.ndarray:
    from concourse.bass_utils import run_bass_kernel_spmd

    in_maps = _prep_in_maps(inputs)

    if "nc" not in _CACHE:
        _CACHE["nc"] = _build_program()
    nc = _CACHE["nc"]

    res = run_bass_kernel_spmd(nc, in_maps, list(range(NCORES))).results
    return _unshard(res)


if __name__ == "__main__":
    # smoke test against the local reference
    sys.path.insert(0, "/root/problem")
    import reference

    inp = reference.setup_inputs()
    expected = np.asarray(reference.reference(**inp))
    actual = kernel(**{k: np.asarray(v) for k, v in inp.items()})
    err = np.linalg.norm(actual - expected) / np.linalg.norm(expected)
    print("Relative error:", err)
